# revision 1
# baseline (speedup 1.0000x reference)
"""BiLSTM (eval-mode, dropout inactive) Trainium2 kernel — 8 NeuronCores.

Problem: x [64, 512, 1024] f32; forward + backward LSTM (H=1024) over
S=512 steps; output [64, 512, 2048] f32.

Sharding: pure data-parallel. Cores 0-3 run the forward LSTM, cores 4-7
the backward LSTM (on time-reversed input); within each direction the
batch (64) is split into 4 quarters of 16. Each core holds its full
per-direction weights and runs the whole recurrence for its batch
quarter — no cross-core communication (measured remote-DMA latency on
this fabric, ~13-16 us/hop, makes per-step tensor-parallel exchange
slower than streaming the full Whh per core).

Per core, one SPMD program, two phases:
  1. pre^T[gate, token] = Wih^T x^T + (bih + bhh), one big GEMM
     (tokens = S*16), staged to DRAM in bf16.
  2. 512 sequential steps: gates^T = Whh^T h^T (+ pre via DVE add),
     sigmoid/tanh on ScalarE, cell update on VectorE. Weights/h in
     bf16 (fp32 PSUM accumulate), cell state c in fp32.
Gate columns are pre-permuted host-side to [i_q f_q o_q g_q] blocks of
128 so sigmoid/tanh each run on contiguous slices. h^T [1024, 16] per
step is staged to DRAM; the host assembles the final output.
"""
import sys

sys.path.insert(0, "/opt/trn_rl_repo")

import numpy as np
import ml_dtypes

from concourse import bass, bacc, tile, bass_utils

mybir = bass.mybir
BF16 = mybir.dt.bfloat16
F32 = mybir.dt.float32
AF = mybir.ActivationFunctionType

bfloat16 = ml_dtypes.bfloat16

B = 64
S = 512
E = 1024
H = 1024
NCORES = 8
BL = 16                 # batch rows per core (4 quarters per direction)
MT = 32                 # gate-column tiles of 128 (4H / 128)
KT = 8                  # contraction tiles (E == H == 1024)
NQ = 8                  # h sub-blocks of 128 (H / 128)
NPAR = 2                # h^T double buffer
TS = 512                # phase-1 token-tile size
KB = KT * BL

TRACE = False           # set True (e.g. from test.py) to capture NTFF timing
LAST_EXEC_NS = None

_cache = {}


def _build_program():
    nc = bacc.Bacc("TRN2", target_bir_lowering=False, debug=False,
                   num_devices=NCORES)
    NT = S * BL // TS

    xT_d = nc.dram_tensor("xT", [E, S * BL], BF16, kind="ExternalInput")
    wih_d = nc.dram_tensor("wih", [128, KT * MT * 128], BF16, kind="ExternalInput")
    whh_d = nc.dram_tensor("whh", [128, KT * MT * 128], BF16, kind="ExternalInput")
    bias_d = nc.dram_tensor("bias", [128, MT], F32, kind="ExternalInput")
    stage_d = nc.dram_tensor("stage", [S, 128, NQ, BL], BF16, kind="ExternalOutput")
    pre_d = nc.dram_tensor("pre_stage", [MT, 128, S, BL], BF16, kind="Internal")

    with tile.TileContext(nc) as tc:
        with (
            tc.tile_pool(name="persist", bufs=1) as persist,
            tc.tile_pool(name="pre", bufs=2) as prep,
            tc.tile_pool(name="ew", bufs=3) as ewp,
        ):
            wih_sb = persist.tile([128, KT * MT * 128], BF16)
            whh_sb = persist.tile([128, KT * MT * 128], BF16)
            bias_sb = persist.tile([128, MT], F32)
            hT = persist.tile([128, NPAR * KB], BF16)
            c_sb = persist.tile([128, 2 * NQ * BL], F32)

            nc.sync.dma_start(wih_sb[:], wih_d[:])
            nc.sync.dma_start(whh_sb[:], whh_d[:])
            nc.sync.dma_start(bias_sb[:], bias_d[:])

            # ---------------- Phase 1: input projection ----------------
            with (
                tc.tile_pool(name="xt", bufs=2) as xtp,
                tc.tile_pool(name="p1psum", bufs=8, space="PSUM") as p1psum,
                tc.tile_pool(name="p1ev", bufs=8) as p1ev,
            ):
                SPT = TS // BL
                for n in range(NT):
                    xt = xtp.tile([128, KT, TS], BF16)
                    for k in range(KT):
                        nc.sync.dma_start(
                            xt[:, k, :],
                            xT_d[k * 128:(k + 1) * 128, n * TS:(n + 1) * TS])
                    for m in range(MT):
                        ps = p1psum.tile([128, TS], F32)
                        for k in range(KT):
                            nc.tensor.matmul(
                                ps[:],
                                wih_sb[:, (k * MT + m) * 128:(k * MT + m + 1) * 128],
                                xt[:, k, :],
                                start=(k == 0), stop=(k == KT - 1))
                        ev = p1ev.tile([128, TS], BF16)
                        nc.scalar.activation(ev[:], ps[:], AF.Identity,
                                             bias=bias_sb[:, m:m + 1], scale=1.0)
                        nc.sync.dma_start(
                            pre_d[m, :, n * SPT:(n + 1) * SPT, :], ev[:])

            # ---------------- Phase 2: recurrence ----------------
            with tc.tile_pool(name="p2psum", bufs=8, space="PSUM") as p2psum:
                pb = None
                for t in range(S):
                    par = t % NPAR
                    par1 = (t - 1) % NPAR
                    cpo = (t - 1) % 2
                    cpn = t % 2
                    tt = t % 8
                    if tt == 0:
                        pb = prep.tile([128, MT, 8, BL], BF16)
                        for m in range(MT):
                            nc.sync.dma_start(pb[:, m, :, :],
                                              pre_d[m, :, t:t + 8, :])

                    qps = []
                    if t > 0:
                        for q in range(NQ):
                            ps = p2psum.tile([128, 4 * BL], F32)
                            for mi in range(4):
                                m = q * 4 + mi
                                for kap in range(KT):
                                    nc.tensor.matmul(
                                        ps[:, mi * BL:(mi + 1) * BL],
                                        whh_sb[:, (kap * MT + m) * 128:
                                               (kap * MT + m + 1) * 128],
                                        hT[:, par1 * KB + kap * BL:
                                           par1 * KB + (kap + 1) * BL],
                                        start=(kap == 0), stop=(kap == KT - 1))
                            qps.append(ps)

                    for q in range(NQ):
                        if t > 0:
                            g = ewp.tile([128, 4 * BL], BF16, tag="g")
                            nc.vector.tensor_add(g[:], qps[q][:],
                                                 pb[:, q * 4:q * 4 + 4, tt, :])
                            g_sig = g[:, 0:3 * BL]
                            g_tanh = g[:, 3 * BL:4 * BL]
                        else:
                            g_sig = pb[:, q * 4:q * 4 + 3, tt, :]
                            g_tanh = pb[:, q * 4 + 3, tt, :]
                        sig = ewp.tile([128, 3 * BL], BF16, tag="sig")
                        nc.scalar.activation(sig[:], g_sig, AF.Sigmoid)
                        tg = ewp.tile([128, BL], BF16, tag="tg")
                        nc.scalar.activation(tg[:], g_tanh, AF.Tanh)

                        c_new = c_sb[:, (cpn * NQ + q) * BL:(cpn * NQ + q + 1) * BL]
                        if t > 0:
                            c_old = c_sb[:, (cpo * NQ + q) * BL:
                                         (cpo * NQ + q + 1) * BL]
                            t1 = ewp.tile([128, BL], F32, tag="t1")
                            nc.vector.tensor_mul(t1[:], sig[:, 0:BL], tg[:])
                            t2 = ewp.tile([128, BL], F32, tag="t2")
                            nc.vector.tensor_mul(t2[:], sig[:, BL:2 * BL], c_old)
                            nc.vector.tensor_add(c_new, t1[:], t2[:])
                        else:
                            nc.vector.tensor_mul(c_new, sig[:, 0:BL], tg[:])
                        tc_ = ewp.tile([128, BL], BF16, tag="tc")
                        nc.scalar.activation(tc_[:], c_new, AF.Tanh)
                        nc.vector.tensor_mul(
                            hT[:, par * KB + q * BL:par * KB + (q + 1) * BL],
                            sig[:, 2 * BL:3 * BL], tc_[:])

                    nc.sync.dma_start(stage_d[t],
                                      hT[:, par * KB:par * KB + NQ * BL])

    nc.compile()
    return nc


def _host_inputs(x, Wih_f, bih_f, Whh_f, bhh_f, Wih_b, bih_b, Whh_b, bhh_b):
    # gate-column permutation: NQ blocks q of [i_q f_q o_q g_q] x 128
    # (reference gate order along 4H is [i, f, g, o])
    cols = []
    for q in range(NQ):
        for goff in (0, H, 3 * H, 2 * H):   # i, f, o, g
            s0 = goff + q * 128
            cols.extend(range(s0, s0 + 128))
    cols = np.array(cols)

    def tiles(w):
        return np.ascontiguousarray(
            w.reshape(KT, 128, MT, 128).transpose(1, 0, 2, 3)
            .reshape(128, KT * MT * 128)).astype(bfloat16)

    per_dir = {}
    for fwd, (Wih, bih, Whh, bhh) in (
            (True, (Wih_f, bih_f, Whh_f, bhh_f)),
            (False, (Wih_b, bih_b, Whh_b, bhh_b))):
        per_dir[fwd] = (
            tiles(Wih[:, cols]),
            tiles(Whh[:, cols]),
            np.ascontiguousarray(
                (bih + bhh)[cols].reshape(MT, 128).T).astype(np.float32),
        )

    in_maps = []
    for c in range(NCORES):
        fwd = c < 4
        qb = c & 3
        xs = x[qb * BL:(qb + 1) * BL]
        if not fwd:
            xs = xs[:, ::-1]
        xT = np.ascontiguousarray(
            xs.transpose(2, 1, 0).reshape(E, S * BL)).astype(bfloat16)
        wih_t, whh_t, bias_t = per_dir[fwd]
        in_maps.append({"xT": xT, "wih": wih_t, "whh": whh_t, "bias": bias_t})
    return in_maps


def _assemble(results):
    out = np.empty((B, S, 2 * H), np.float32)
    for c in range(NCORES):
        fwd = c < 4
        qb = c & 3
        arr = np.asarray(results[c]["stage"]).astype(np.float32)
        part = arr.transpose(3, 0, 2, 1).reshape(BL, S, NQ * 128)
        if not fwd:
            part = part[:, ::-1, :]
        base = 0 if fwd else H
        out[qb * BL:(qb + 1) * BL, :, base:base + H] = part
    return out


def kernel(x, Wih_f, bih_f, Whh_f, bhh_f, Wih_b, bih_b, Whh_b, bhh_b):
    global LAST_EXEC_NS
    if "nc" not in _cache:
        _cache["nc"] = _build_program()
    nc = _cache["nc"]
    in_maps = _host_inputs(np.asarray(x, np.float32),
                           np.asarray(Wih_f, np.float32),
                           np.asarray(bih_f, np.float32),
                           np.asarray(Whh_f, np.float32),
                           np.asarray(bhh_f, np.float32),
                           np.asarray(Wih_b, np.float32),
                           np.asarray(bih_b, np.float32),
                           np.asarray(Whh_b, np.float32),
                           np.asarray(bhh_b, np.float32))
    res = bass_utils.run_bass_kernel_spmd(nc, in_maps,
                                          core_ids=list(range(NCORES)),
                                          trace=TRACE)
    LAST_EXEC_NS = res.exec_time_ns
    return _assemble(res.results)
